# revision 1
# baseline (speedup 1.0000x reference)
"""DenseSNN Trainium2 kernel: 4-layer LIF SNN, T=100 steps, B=128, D=H=2048, C=100.

Strategy
--------
The reference scans timesteps with all 4 layers inside the scan body, but the
dependency structure is feed-forward across layers: layer-l spikes at step t
depend only on layer-(l-1) spikes at steps <= t. So the computation unrolls into
per-layer phases:

    CUR1 = x @ W1 + b1          (batched over all T*B rows)
    S1   = LIF-scan_T(CUR1)     (elementwise in (B,H), sequential in T)
    CUR2 = S1 @ W2 + b2 ; S2 = LIF-scan(CUR2)
    CUR3 = S2 @ W3 + b3 ; S3 = LIF-scan(CUR3)
    CURo = S3 @ Wo + bo ; out = sum_t LIF-scan(CURo)

This turns the tiny per-step GEMMs into full-size GEMMs and makes pure
data-parallelism over batch (16 samples/core on 8 cores) communication-free.

On-chip layout is "transposed activations": [feature -> 16 chunks x 128
partitions, (t,b) -> free axis]. Weight-stationary matmuls (lhsT = W tile in
natural [D,H] layout) keep every tensor in this layout end to end; the host
pre-transposes x and re-assembles the output, so the device never transposes.

Matmuls run in bf16 (inputs cast on host) with fp32 PSUM accumulation; LIF
membrane state is fp32 on the vector engine. Spikes are exactly representable
in bf16. reset(t) == spike(t-1), which saves one compare per step.
"""

import numpy as np
import ml_dtypes

import concourse.bass as bass
import concourse.mybir as mybir
import concourse.tile as tile
from concourse import bacc
from concourse.bass_utils import run_bass_kernel_spmd

# Problem constants (hardcoded per contract)
T, B, D, H, C = 100, 128, 2048, 2048, 100
NCORES = 8
BC = B // NCORES          # 16 samples per core
R = T * BC                # 1600 rows (t,b) per core
KC = D // 128             # 16 contraction chunks
HC = H // 128             # 16 output-feature chunks
BETA = 0.9
NR = 256                  # row-slice width (multiple of BC)
SLICES = [(r0, min(NR, R - r0)) for r0 in range(0, R, NR)]

import os
_DEBUG_SPIKES = bool(os.environ.get("SNN_DEBUG_SPIKES"))
F32 = mybir.dt.float32
BF16 = mybir.dt.bfloat16
ALU = mybir.AluOpType
ACTF = mybir.ActivationFunctionType


def _build_nc():
    nc = bacc.Bacc("TRN2", target_bir_lowering=False)

    xT_d = nc.dram_tensor("xT", [KC, 128, R], BF16, kind="ExternalInput")
    w_d = [
        nc.dram_tensor("w1", [D, H], BF16, kind="ExternalInput"),
        nc.dram_tensor("w2", [H, H], BF16, kind="ExternalInput"),
        nc.dram_tensor("w3", [H, H], BF16, kind="ExternalInput"),
    ]
    wo_d = nc.dram_tensor("wo", [H, C], BF16, kind="ExternalInput")
    bias_d = nc.dram_tensor("biases", [128, 3 * HC], F32, kind="ExternalInput")
    bo_d = nc.dram_tensor("biaso", [C, 1], F32, kind="ExternalInput")
    out_d = nc.dram_tensor("out", [C, BC], F32, kind="ExternalOutput")

    with tile.TileContext(nc) as tc:
        with (
            tc.tile_pool(name="spool", bufs=2) as spool,
            tc.tile_pool(name="wpool", bufs=1) as wpool,
            tc.tile_pool(name="stream", bufs=3) as stream,
            tc.tile_pool(name="small", bufs=1) as small,
            tc.tile_pool(name="pspool", bufs=8, space="PSUM") as pspool,
        ):
            # Persistent big tensors
            S1 = spool.tile([128, KC * R], BF16, tag="S")
            S2 = spool.tile([128, KC * R], BF16, tag="S")
            S3 = spool.tile([128, KC * R], BF16, tag="S")  # reuses S1's slot
            w_sb = [
                wpool.tile([128, KC * H], BF16, tag="W", name=f"w{i}_sb")
                for i in range(3)
            ]
            wo_sb = small.tile([128, KC * C], BF16)

            # Small state: fp32 [128, 1024] packs mems/biases/output-layer state
            st = small.tile([128, 1152], F32)
            mem = [
                st[:, 0:256].rearrange("p (c b) -> p c b", c=KC),
                st[:, 256:512].rearrange("p (c b) -> p c b", c=KC),
                st[:, 512:768].rearrange("p (c b) -> p c b", c=KC),
            ]
            bias_sb = st[:, 768:816]            # [128, 48] = 3 layers x 16 chunks
            memo = st[:100, 816:832]            # [100, 16]
            ssum = st[:100, 832:848]
            zo = st[:100, 848:864]              # zeros (Lo t=0 s_prev)
            so_ring = st[:100, 864:896]         # [100, 32] ping-pong spikes
            bo_sb = st[:100, 896:897]           # [100, 1]
            zeros_bf = small.tile([128, 256], BF16)
            z3 = zeros_bf.rearrange("p (c b) -> p c b", c=KC)

            nc.gpsimd.memset(st[:], 0.0)
            nc.gpsimd.memset(zeros_bf[:], 0.0)
            nc.sync.dma_start(bias_sb, bias_d[:])
            nc.sync.dma_start(bo_sb, bo_d[:])
            for kc in range(KC):
                nc.sync.dma_start(
                    wo_sb[:, kc * C:(kc + 1) * C], wo_d[kc * 128:(kc + 1) * 128, :]
                )

            def dense_layer(li, rhs_of, S_out):
                """One hidden layer: matmul all row-slices + LIF scan over T."""
                w = w_sb[li]
                for kc in range(KC):
                    nc.sync.dma_start(
                        w[:, kc * H:(kc + 1) * H],
                        w_d[li][kc * 128:(kc + 1) * 128, :],
                    )
                S_out3 = S_out.rearrange("p (c r) -> p c r", c=KC)
                m3 = mem[li]
                for r0, nr in SLICES:
                    rhs = rhs_of(r0, nr)
                    cur = stream.tile([128, KC * NR], BF16, tag="stream", name="cur")
                    for hc in range(HC):
                        ps = pspool.tile([128, NR], F32, tag="ps", name="ps")
                        for kc in range(KC):
                            nc.tensor.matmul(
                                ps[:, :nr],
                                w[:, kc * H + hc * 128: kc * H + hc * 128 + 128],
                                rhs(kc),
                                start=(kc == 0),
                                stop=(kc == KC - 1),
                            )
                        nc.scalar.activation(
                            cur[:, hc * nr:(hc + 1) * nr],
                            ps[:, :nr],
                            ACTF.Identity,
                            bias=bias_sb[:, li * HC + hc: li * HC + hc + 1],
                            scale=1.0,
                        )
                    cur3 = cur[:, : KC * nr].rearrange("p (c r) -> p c r", c=KC)
                    for tl in range(nr // BC):
                        t = r0 // BC + tl
                        cur_t = cur3[:, :, tl * BC:(tl + 1) * BC]
                        s_prev = (
                            S_out3[:, :, (t - 1) * BC: t * BC] if t > 0 else z3
                        )
                        s_new = S_out3[:, :, t * BC:(t + 1) * BC]
                        # tmp = beta*mem + cur
                        nc.vector.scalar_tensor_tensor(
                            m3, m3, BETA, cur_t, ALU.mult, ALU.add
                        )
                        # spike = (tmp - 1) > s_prev   (== mem_new > 1)
                        nc.vector.scalar_tensor_tensor(
                            s_new, m3, 1.0, s_prev, ALU.subtract, ALU.is_gt
                        )
                        # mem_new = tmp - s_prev
                        nc.vector.tensor_tensor(m3, m3, s_prev, ALU.subtract)

            # ---- Layer 1: rhs streamed from HBM (x^T, host-pretransposed)
            def rhs_layer1(r0, nr):
                xin = stream.tile([128, KC * NR], BF16, tag="stream", name="xin")
                for kc in range(KC):
                    nc.sync.dma_start(
                        xin[:, kc * nr:(kc + 1) * nr], xT_d[kc][:, r0:r0 + nr]
                    )
                return lambda kc: xin[:, kc * nr:(kc + 1) * nr]

            dense_layer(0, rhs_layer1, S1)

            # ---- Layers 2, 3: rhs from previous layer's spikes in SBUF
            def rhs_from(S_in):
                S_in3 = S_in.rearrange("p (c r) -> p c r", c=KC)
                def f(r0, nr):
                    return lambda kc: S_in3[:, kc, r0:r0 + nr]
                return f

            dense_layer(1, rhs_from(S1), S2)
            dense_layer(2, rhs_from(S2), S3)

            # ---- Output layer + spike-count accumulation
            S3_3 = S3.rearrange("p (c r) -> p c r", c=KC)
            for r0, nr in SLICES:
                ps = pspool.tile([128, NR], F32, tag="ps", name="pso")
                for kc in range(KC):
                    nc.tensor.matmul(
                        ps[:100, :nr],
                        wo_sb[:, kc * C:(kc + 1) * C],
                        S3_3[:, kc, r0:r0 + nr],
                        start=(kc == 0),
                        stop=(kc == KC - 1),
                    )
                curo = stream.tile([128, NR], F32, tag="stream", name="curo")
                curo_f = curo[:100, :nr]
                nc.scalar.activation(
                    curo_f, ps[:100, :nr], ACTF.Identity,
                    bias=bo_sb, scale=1.0,
                )
                for tl in range(nr // BC):
                    t = r0 // BC + tl
                    cur_t = curo_f[:, tl * BC:(tl + 1) * BC]
                    so_prev = zo if t == 0 else so_ring[:, (1 - t % 2) * BC:(2 - t % 2) * BC]
                    so_new = so_ring[:, (t % 2) * BC:(t % 2 + 1) * BC]
                    nc.vector.scalar_tensor_tensor(
                        memo, memo, BETA, cur_t, ALU.mult, ALU.add
                    )
                    nc.vector.scalar_tensor_tensor(
                        so_new, memo, 1.0, so_prev, ALU.subtract, ALU.is_gt
                    )
                    nc.vector.tensor_tensor(memo, memo, so_prev, ALU.subtract)
                    nc.vector.tensor_tensor(ssum, ssum, so_new, ALU.add)

            nc.sync.dma_start(out_d[:], ssum)

            if _DEBUG_SPIKES:
                for nm, S in (("s1_dbg", S1), ("s2_dbg", S2), ("s3_dbg", S3)):
                    sd = nc.dram_tensor(nm, [128, KC * R], BF16, kind="ExternalOutput")
                    nc.sync.dma_start(sd[:], S[:])

    nc.compile()
    return nc


_NC_CACHE = None


def _get_nc():
    global _NC_CACHE
    if _NC_CACHE is None:
        _NC_CACHE = _build_nc()
    return _NC_CACHE


def make_in_maps(x_seq, W1, b1, W2, b2, W3, b3, Wo, bo):
    bf = ml_dtypes.bfloat16
    w1 = np.ascontiguousarray(W1.astype(bf))
    w2 = np.ascontiguousarray(W2.astype(bf))
    w3 = np.ascontiguousarray(W3.astype(bf))
    wo = np.ascontiguousarray(Wo.astype(bf))
    biases = np.concatenate(
        [b.reshape(HC, 128).T for b in (b1, b2, b3)], axis=1
    ).astype(np.float32)                       # [128, 48]
    biases = np.ascontiguousarray(biases)
    bo_a = np.ascontiguousarray(bo.reshape(C, 1).astype(np.float32))
    in_maps = []
    for c in range(NCORES):
        xs = x_seq[:, c * BC:(c + 1) * BC, :]              # [T, BC, D]
        xT = xs.transpose(2, 0, 1).reshape(KC, 128, R)     # [D,(t,b)] chunked
        in_maps.append({
            "xT": np.ascontiguousarray(xT.astype(bf)),
            "w1": w1, "w2": w2, "w3": w3, "wo": wo,
            "biases": biases, "biaso": bo_a,
        })
    return in_maps


def kernel(x_seq, W1, b1, W2, b2, W3, b3, Wo, bo):
    nc = _get_nc()
    in_maps = make_in_maps(x_seq, W1, b1, W2, b2, W3, b3, Wo, bo)
    res = run_bass_kernel_spmd(nc, in_maps, core_ids=list(range(NCORES)))
    outs = [res.results[c]["out"] for c in range(NCORES)]   # each [C, BC]
    return np.concatenate([o.T for o in outs], axis=0).astype(np.float32)

